# revision 24
# baseline (speedup 1.0000x reference)
"""Trainium2 Bass kernel for a multi-head self-attention block.

Reference computation (shapes hardcoded):
    x: (4, 256, 2048) f32, w_qkv: (1536, 256), w_out: (256, 512), b_out: (256,)
    qkv = w_qkv @ x ; q,k,v heads of dim 64 (8 heads); q *= 64**-0.5
    sim = q^T k per (b, h); attn = softmax(sim, axis=-1)
    out = w_out @ (attn @ v^T rearranged) + b_out

Sharding: 8 cores = 4 batches x 2 head-groups (4 heads each). Each core
computes its batch's qkv projection for its heads, attention, and a partial
output projection over its heads' hidden columns. Host sums the two partial
outputs per batch and adds b_out.

Per-core layout choices:
  - scores are computed TRANSPOSED: S_T[j, i] = sum_d k[d,j] q[d,i] via
    matmul(lhsT=k_head, rhs=q_head) so that E = exp(S_T) (ScalarE, straight
    from PSUM) is directly the moving operand of attn@v.
  - attn@v uses lhsT = [v^T | ones] (128 j-rows x 65 cols) so PSUM row 64
    accumulates the softmax denominator alongside the 64 output dims.
  - v^T is produced directly by the qkv projection (lhsT=x, rhs=w_v^T), no
    transposes anywhere in the kernel.
  - normalization happens after attn@v on (4*64, 2048) values: batched
    reciprocal of the 16 denominator rows, PE ones-matmul broadcast across
    partitions, one tensor_tensor multiply.
All matmuls run in float32r (full PE rate for free dim >= 256, fp32 results).
"""

import numpy as np

import concourse.bass as bass
import concourse.mybir as mybir
import concourse.tile as tile
from concourse import bacc
from concourse.bass_utils import run_bass_kernel_spmd

B, C, N = 4, 256, 2048
HEADS = 8  # total; 4 per core
DH = 64
HID = HEADS * DH  # 512
SCALE = DH ** -0.5
NCORES = 8
HPC = 4  # heads per core

F32 = mybir.dt.float32
F32R = mybir.dt.float32r

_CACHED_NC = None


def r(ap):
    """View an AP as float32r for full-rate PE matmuls."""
    if ap.dtype == F32R:
        return ap
    return ap.bitcast(F32R)


def _emit(nc):
    # ---- DRAM parameters (per-core shard) ----
    x_d = nc.declare_dram_parameter("x", [2, 128, N], F32R, isOutput=False)
    wqk_d = nc.declare_dram_parameter("wqkT", [2, 128, 512], F32R, isOutput=False)
    wv_d = nc.declare_dram_parameter("wvT", [2, 128, 256], F32R, isOutput=False)
    wo_d = nc.declare_dram_parameter("woutT", [2, 128, 256], F32R, isOutput=False)
    # one-hot row-gather (1, 16*16) and per-(ic,pt) partition-broadcast
    # selectors (16, 8, 128); see phase 3.
    oneh_d = nc.declare_dram_parameter("onehot", [1, 256], F32, isOutput=False)
    sel_d = nc.declare_dram_parameter("sel16", [16, 8, 128], F32, isOutput=False)
    ones_d = nc.declare_dram_parameter("ones", [128, 64], F32R, isOutput=False)
    out_d = nc.declare_dram_parameter("out", [2, 128, N], F32, isOutput=True)

    with tile.TileContext(nc) as tc:
        with (
            tc.tile_pool(name="const", bufs=1) as const,
            tc.tile_pool(name="epool", bufs=3) as epool,
            tc.tile_pool(name="ps_sc", bufs=2, space="PSUM") as ps_sc,
            tc.tile_pool(name="ps_av", bufs=2, space="PSUM") as ps_av,
            tc.tile_pool(name="ps_misc", bufs=1, space="PSUM") as ps_misc,
        ):
            # ---- persistent SBUF tiles ----
            x_sb = const.tile([128, 2, N], F32R)            # x, c on partitions
            wqk_sb = const.tile([128, 2, 512], F32R)        # (c, [q|k] rows)
            wv_sb = const.tile([128, 2, 256], F32R)         # (c, v^T cols)
            wo_sb = const.tile([128, 2, 256], F32R)         # (hd, c_out)
            oneh_sb = const.tile([1, 256], F32)
            sel_sb = const.tile([16, 8, 128], F32)         # bcast selectors
            qk_sb = const.tile([128, 4, N], F32R)           # mt: qh01,qh23,kh01,kh23
            vt_sb = const.tile([128, 16, HPC, DH + 1], F32R)  # (j, jt, h, d|ones)
            attn_sb = const.tile([128, 2, N], F32R)         # attn@v out (hd, i)
            den_free = const.tile([1, 16, 512], F32)       # slot = 4*ic + h
            den_sb = const.tile([16, 512], F32)            # row = 4*ic + h
            rec_sb = const.tile([16, 512], F32)
            out_sb = const.tile([128, 2, N], F32)

            for kt in range(2):
                nc.sync.dma_start(out=x_sb[:, kt, :], in_=x_d[kt])
                nc.sync.dma_start(out=wqk_sb[:, kt, :], in_=wqk_d[kt])
                nc.sync.dma_start(out=wv_sb[:, kt, :], in_=wv_d[kt])
                nc.sync.dma_start(out=wo_sb[:, kt, :], in_=wo_d[kt])
            nc.sync.dma_start(out=sel_sb[:], in_=sel_d[:])
            nc.sync.dma_start(out=oneh_sb[:], in_=oneh_d[:])

            # ones column of v^T-augmented tiles (DMA keeps the f32r dtype
            # chain intact for the BIR verifier; memset can't target f32r)
            nc.sync.dma_start(
                out=vt_sb[:, :, :, DH],
                in_=ones_d.rearrange("p (a b) -> p a b", a=16),
            )

            # ---- phase 1: qkv projection ----
            # q,k rows: out = wqkT.T @ x, M-tiles over 512 rows
            for mt in range(4):
                for icg in range(2):  # groups of 2 i-chunks
                    pq = ps_sc.tile([128, 2, 512], F32, name="ps_sc", tag="ps_sc")
                    for u in range(2):
                        base = icg * 1024 + u * 512
                        for kt in range(2):
                            nc.tensor.matmul(
                                pq[:, u, :],
                                r(wqk_sb[:, kt, mt * 128:(mt + 1) * 128]),
                                r(x_sb[:, kt, base:base + 512]),
                                start=(kt == 0), stop=(kt == 1),
                            )
                    nc.vector.tensor_copy(
                        out=qk_sb[:, mt, icg * 1024:(icg + 1) * 1024].rearrange(
                            "p (a f) -> p a f", a=2),
                        in_=pq[:],
                    )

            # v^T: out[n-tile, h*d] = x.T @ wvT
            for nt in range(16):
                pv = ps_misc.tile([128, 256], F32, name="ps_misc", tag="ps_misc")
                for kt in range(2):
                    nc.tensor.matmul(
                        pv[:],
                        r(x_sb[:, kt, nt * 128:(nt + 1) * 128]),
                        r(wv_sb[:, kt, :]),
                        start=(kt == 0), stop=(kt == 1),
                    )
                nc.vector.tensor_copy(
                    out=vt_sb[:, nt, :, 0:DH],
                    in_=pv.rearrange("p (h d) -> p h d", h=HPC),
                )

            # ---- phase 2: attention per (head, i-chunk) ----
            for h in range(HPC):
                po = 64 * (h % 2)          # partition offset within M-tile
                mt_q = h // 2              # q M-tile index
                mt_k = 2 + h // 2          # k M-tile index
                for ic in range(4):
                    pav = ps_av.tile([DH + 1, 512], F32, name="ps_av")
                    for half in range(2):
                        e_sb = epool.tile([128, 8, 512], F32R, name="e", tag="e")
                        for g in range(4):  # groups of 2 j-tiles
                            psc = ps_sc.tile([128, 2, 512], F32, name="ps_sc",
                                             tag="ps_sc")
                            for u in range(2):
                                jt = half * 8 + 2 * g + u
                                nc.tensor.matmul(
                                    psc[:, u, :],
                                    r(qk_sb[po:po + 64, mt_k,
                                            jt * 128:(jt + 1) * 128]),
                                    r(qk_sb[po:po + 64, mt_q,
                                            ic * 512:(ic + 1) * 512]),
                                    start=True, stop=True,
                                )
                            nc.scalar.activation(
                                e_sb[:, 2 * g:2 * g + 2, :], psc[:],
                                mybir.ActivationFunctionType.Exp,
                            )
                        for j in range(8):
                            nc.tensor.matmul(
                                pav[:],
                                r(vt_sb[:, half * 8 + j, h, :]),
                                r(e_sb[:, j, :]),
                                start=(half == 0 and j == 0),
                                stop=(half == 1 and j == 7),
                            )
                    nc.vector.tensor_copy(
                        out=attn_sb[po:po + 64, h // 2, ic * 512:(ic + 1) * 512],
                        in_=pav[0:DH, :],
                    )
                    nc.vector.tensor_copy(
                        out=den_free[0:1, 4 * ic + h, :],
                        in_=pav[DH:DH + 1, :],
                    )

            # ---- phase 3: normalize ----
            # gather the 16 denominator rows (partition 0, free slots) onto 16
            # partitions via one-hot K=1 matmuls, then one batched reciprocal
            pden = ps_misc.tile([16, 512], F32, name="ps_misc", tag="ps_misc")
            for i in range(16):
                nc.tensor.matmul(
                    pden[:],
                    oneh_sb[0:1, i * 16:(i + 1) * 16],
                    den_free[0:1, i, :],
                    start=(i == 0), stop=(i == 15),
                )
            nc.vector.tensor_copy(out=den_sb[:], in_=pden[:])
            nc.vector.reciprocal(rec_sb[:], den_sb[:])
            for ic in range(4):
                for pt in range(2):
                    pb = ps_misc.tile([128, 512], F32, name="ps_misc", tag="ps_misc")
                    nc.tensor.matmul(
                        pb[:], sel_sb[:, ic * 2 + pt, :], rec_sb[:],
                        start=True, stop=True,
                    )
                    nc.vector.tensor_tensor(
                        attn_sb[:, pt, ic * 512:(ic + 1) * 512],
                        attn_sb[:, pt, ic * 512:(ic + 1) * 512],
                        pb[:],
                        mybir.AluOpType.mult,
                    )

            # ---- phase 4: output projection (partial, this core's heads) ----
            for mt in range(2):
                for icg in range(2):  # groups of 2 i-chunks
                    pp = ps_sc.tile([128, 2, 512], F32, name="ps_sc", tag="ps_sc")
                    for u in range(2):
                        ic = 2 * icg + u
                        for kt in range(2):
                            nc.tensor.matmul(
                                pp[:, u, :],
                                r(wo_sb[:, kt, mt * 128:(mt + 1) * 128]),
                                r(attn_sb[:, kt, ic * 512:(ic + 1) * 512]),
                                start=(kt == 0), stop=(kt == 1),
                            )
                    nc.vector.tensor_copy(
                        out=out_sb[:, mt, icg * 1024:(icg + 1) * 1024].rearrange(
                            "p (a f) -> p a f", a=2),
                        in_=pp[:],
                    )
                    nc.sync.dma_start(
                        out=out_d[mt, :, icg * 1024:(icg + 1) * 1024],
                        in_=out_sb[:, mt, icg * 1024:(icg + 1) * 1024],
                    )

    return nc


def build_nc():
    global _CACHED_NC
    if _CACHED_NC is None:
        # Bacc (not plain Bass): its compile() runs
        # move_matmul_waits_to_ldweights + generate_event_semaphores, which
        # split multi-wait instructions that walrus codegen rejects.
        nc = bacc.Bacc(None)
        _emit(nc)
        if not nc.is_finalized():
            nc.finalize()
        _CACHED_NC = nc
    return _CACHED_NC


def make_core_inputs(x, w_qkv, w_out):
    """Host-side shard prep. Returns list of per-core input dicts."""
    x = np.asarray(x, np.float32)
    w_qkv = np.asarray(w_qkv, np.float32)
    w_out = np.asarray(w_out, np.float32)
    wq = w_qkv[0:HID] * np.float32(SCALE)   # (512, 256), head h rows h*64:...
    wk = w_qkv[HID:2 * HID]
    wv = w_qkv[2 * HID:3 * HID]
    onehot = np.zeros((1, 256), np.float32)
    for i in range(16):
        onehot[0, i * 16 + i] = 1.0
    sel16 = np.zeros((16, 8, 128), np.float32)
    for ic in range(4):
        for pt in range(2):
            for p in range(128):
                sel16[4 * ic + 2 * pt + (p >= 64), ic * 2 + pt, p] = 1.0

    in_maps = []
    for core in range(NCORES):
        b, g = divmod(core, 2)
        hs = slice(g * HPC * DH, (g + 1) * HPC * DH)  # 256 hidden cols/rows
        wqk = np.concatenate([wq[hs], wk[hs]], axis=0)          # (512, 256)
        wqkT = np.ascontiguousarray(wqk.T).reshape(2, 128, 512)
        wvT = np.ascontiguousarray(wv[hs].T).reshape(2, 128, 256)
        woT = np.ascontiguousarray(w_out[:, hs].T).reshape(2, 128, 256)
        in_maps.append({
            "x": np.ascontiguousarray(x[b]).reshape(2, 128, N),
            "wqkT": wqkT,
            "wvT": wvT,
            "woutT": woT,
            "onehot": onehot,
            "sel16": sel16,
            "ones": np.ones((128, 64), np.float32),
        })
    return in_maps


def kernel(x, w_qkv, w_out, b_out, trace=False):
    nc = build_nc()
    in_maps = make_core_inputs(x, w_qkv, w_out)
    res = run_bass_kernel_spmd(nc, in_maps, core_ids=list(range(NCORES)),
                               trace=trace)
    out = np.empty((B, C, N), np.float32)
    b_out = np.asarray(b_out, np.float32)
    for b in range(B):
        p0 = np.asarray(res.results[2 * b]["out"]).reshape(C, N)
        p1 = np.asarray(res.results[2 * b + 1]["out"]).reshape(C, N)
        out[b] = p0 + p1 + b_out[:, None]
    if trace:
        return out, res
    return out


# revision 25
# speedup vs baseline: 1.5863x; 1.5863x over previous
"""Trainium2 Bass kernel for a multi-head self-attention block.

Reference computation (shapes hardcoded):
    x: (4, 256, 2048) f32, w_qkv: (1536, 256), w_out: (256, 512), b_out: (256,)
    qkv = w_qkv @ x ; q,k,v heads of dim 64 (8 heads); q *= 64**-0.5
    sim = q^T k per (b, h); attn = softmax(sim, axis=-1)
    out = w_out @ (attn @ v^T rearranged) + b_out

Sharding: 8 cores = 4 batches x 2 head-groups (4 heads each). Each core
computes its batch's qkv projection for its heads, attention, and a partial
output projection over its heads' hidden columns. Host sums the two partial
outputs per batch and adds b_out.

Per-core layout choices:
  - scores are computed TRANSPOSED: S_T[j, i] = sum_d k[d,j] q[d,i] via
    matmul(lhsT=k_head, rhs=q_head) so that E = exp(S_T) (ScalarE, straight
    from PSUM) is directly the moving operand of attn@v.
  - attn@v uses lhsT = [v^T | ones] (128 j-rows x 65 cols) so PSUM row 64
    accumulates the softmax denominator alongside the 64 output dims.
  - v^T is produced directly by the qkv projection (lhsT=x, rhs=w_v^T), no
    transposes anywhere in the kernel.
  - normalization happens after attn@v on (4*64, 2048) values: batched
    reciprocal of the 16 denominator rows, PE ones-matmul broadcast across
    partitions, one tensor_tensor multiply.
All matmuls run in float32r (full PE rate for free dim >= 256, fp32 results).
"""

import numpy as np

import concourse.bass as bass
import concourse.mybir as mybir
import concourse.tile as tile
from concourse import bacc
from concourse.bass_utils import run_bass_kernel_spmd

B, C, N = 4, 256, 2048
HEADS = 8  # total; 4 per core
DH = 64
HID = HEADS * DH  # 512
SCALE = DH ** -0.5
NCORES = 8
HPC = 4  # heads per core

F32 = mybir.dt.float32
F32R = mybir.dt.float32r
F16 = mybir.dt.float16

_CACHED_NC = None


def r(ap):
    """Matmul operand passthrough (fp16 tiles are consumed directly)."""
    return ap


def _emit(nc):
    # ---- DRAM parameters (per-core shard) ----
    x_d = nc.declare_dram_parameter("x", [2, 128, N], F16, isOutput=False)
    wqk_d = nc.declare_dram_parameter("wqkT", [2, 128, 512], F16, isOutput=False)
    wv_d = nc.declare_dram_parameter("wvT", [2, 128, 256], F16, isOutput=False)
    wo_d = nc.declare_dram_parameter("woutT", [2, 128, 256], F16, isOutput=False)
    # one-hot row-gather (1, 16*16) and per-(ic,pt) partition-broadcast
    # selectors (16, 8, 128); see phase 3.
    oneh_d = nc.declare_dram_parameter("onehot", [1, 256], F32, isOutput=False)
    sel_d = nc.declare_dram_parameter("sel16", [16, 8, 128], F32, isOutput=False)
    ones_d = nc.declare_dram_parameter("ones", [128, 64], F16, isOutput=False)
    out_d = nc.declare_dram_parameter("out", [2, 128, N], F32, isOutput=True)

    with tile.TileContext(nc) as tc:
        with (
            tc.tile_pool(name="const", bufs=1) as const,
            tc.tile_pool(name="epool", bufs=3) as epool,
            tc.tile_pool(name="ps_sc", bufs=2, space="PSUM") as ps_sc,
            tc.tile_pool(name="ps_av", bufs=2, space="PSUM") as ps_av,
            tc.tile_pool(name="ps_misc", bufs=1, space="PSUM") as ps_misc,
        ):
            # ---- persistent SBUF tiles ----
            x_sb = const.tile([128, 2, N], F16)            # x, c on partitions
            wqk_sb = const.tile([128, 2, 512], F16)        # (c, [q|k] rows)
            wv_sb = const.tile([128, 2, 256], F16)         # (c, v^T cols)
            wo_sb = const.tile([128, 2, 256], F16)         # (hd, c_out)
            oneh_sb = const.tile([1, 256], F32)
            sel_sb = const.tile([16, 8, 128], F32)         # bcast selectors
            qk_sb = const.tile([128, 4, N], F16)           # mt: qh01,qh23,kh01,kh23
            vt_sb = const.tile([128, 16, HPC, DH + 1], F16)  # (j, jt, h, d|ones)
            attn_sb = const.tile([128, 2, N], F16)         # attn@v out (hd, i)
            den_free = const.tile([1, 16, 512], F32)       # slot = 4*ic + h
            den_sb = const.tile([16, 512], F32)            # row = 4*ic + h
            rec_sb = const.tile([16, 512], F32)
            out_sb = const.tile([128, 2, N], F32)

            for kt in range(2):
                nc.sync.dma_start(out=x_sb[:, kt, :], in_=x_d[kt])
                nc.sync.dma_start(out=wqk_sb[:, kt, :], in_=wqk_d[kt])
                nc.sync.dma_start(out=wv_sb[:, kt, :], in_=wv_d[kt])
                nc.sync.dma_start(out=wo_sb[:, kt, :], in_=wo_d[kt])
            nc.sync.dma_start(out=sel_sb[:], in_=sel_d[:])
            nc.sync.dma_start(out=oneh_sb[:], in_=oneh_d[:])

            # ones column of v^T-augmented tiles (DMA keeps the f32r dtype
            # chain intact for the BIR verifier; memset can't target f32r)
            nc.sync.dma_start(
                out=vt_sb[:, :, :, DH],
                in_=ones_d.rearrange("p (a b) -> p a b", a=16),
            )

            # ---- phase 1: qkv projection ----
            # q,k rows: out = wqkT.T @ x, M-tiles over 512 rows
            for mt in range(4):
                for icg in range(2):  # groups of 2 i-chunks
                    pq = ps_sc.tile([128, 2, 512], F32, name="ps_sc", tag="ps_sc")
                    for u in range(2):
                        base = icg * 1024 + u * 512
                        for kt in range(2):
                            nc.tensor.matmul(
                                pq[:, u, :],
                                r(wqk_sb[:, kt, mt * 128:(mt + 1) * 128]),
                                r(x_sb[:, kt, base:base + 512]),
                                start=(kt == 0), stop=(kt == 1),
                            )
                    nc.vector.tensor_copy(
                        out=qk_sb[:, mt, icg * 1024:(icg + 1) * 1024].rearrange(
                            "p (a f) -> p a f", a=2),
                        in_=pq[:],
                    )

            # v^T: out[n-tile, h*d] = x.T @ wvT
            for nt in range(16):
                pv = ps_misc.tile([128, 256], F32, name="ps_misc", tag="ps_misc")
                for kt in range(2):
                    nc.tensor.matmul(
                        pv[:],
                        r(x_sb[:, kt, nt * 128:(nt + 1) * 128]),
                        r(wv_sb[:, kt, :]),
                        start=(kt == 0), stop=(kt == 1),
                    )
                nc.vector.tensor_copy(
                    out=vt_sb[:, nt, :, 0:DH],
                    in_=pv.rearrange("p (h d) -> p h d", h=HPC),
                )

            # ---- phase 2: attention per (head, i-chunk) ----
            for h in range(HPC):
                po = 64 * (h % 2)          # partition offset within M-tile
                mt_q = h // 2              # q M-tile index
                mt_k = 2 + h // 2          # k M-tile index
                for ic in range(4):
                    pav = ps_av.tile([DH + 1, 512], F32, name="ps_av")
                    for half in range(2):
                        e_sb = epool.tile([128, 8, 512], F16, name="e", tag="e")
                        for g in range(4):  # groups of 2 j-tiles
                            psc = ps_sc.tile([128, 2, 512], F32, name="ps_sc",
                                             tag="ps_sc")
                            for u in range(2):
                                jt = half * 8 + 2 * g + u
                                nc.tensor.matmul(
                                    psc[:, u, :],
                                    r(qk_sb[po:po + 64, mt_k,
                                            jt * 128:(jt + 1) * 128]),
                                    r(qk_sb[po:po + 64, mt_q,
                                            ic * 512:(ic + 1) * 512]),
                                    start=True, stop=True,
                                )
                            nc.scalar.activation(
                                e_sb[:, 2 * g:2 * g + 2, :], psc[:],
                                mybir.ActivationFunctionType.Exp,
                            )
                        for j in range(8):
                            nc.tensor.matmul(
                                pav[:],
                                r(vt_sb[:, half * 8 + j, h, :]),
                                r(e_sb[:, j, :]),
                                start=(half == 0 and j == 0),
                                stop=(half == 1 and j == 7),
                            )
                    nc.vector.tensor_copy(
                        out=attn_sb[po:po + 64, h // 2, ic * 512:(ic + 1) * 512],
                        in_=pav[0:DH, :],
                    )
                    nc.vector.tensor_copy(
                        out=den_free[0:1, 4 * ic + h, :],
                        in_=pav[DH:DH + 1, :],
                    )

            # ---- phase 3: normalize ----
            # gather the 16 denominator rows (partition 0, free slots) onto 16
            # partitions via one-hot K=1 matmuls, then one batched reciprocal
            pden = ps_misc.tile([16, 512], F32, name="ps_misc", tag="ps_misc")
            for i in range(16):
                nc.tensor.matmul(
                    pden[:],
                    oneh_sb[0:1, i * 16:(i + 1) * 16],
                    den_free[0:1, i, :],
                    start=(i == 0), stop=(i == 15),
                )
            nc.vector.tensor_copy(out=den_sb[:], in_=pden[:])
            nc.vector.reciprocal(rec_sb[:], den_sb[:])
            for ic in range(4):
                for pt in range(2):
                    pb = ps_misc.tile([128, 512], F32, name="ps_misc", tag="ps_misc")
                    nc.tensor.matmul(
                        pb[:], sel_sb[:, ic * 2 + pt, :], rec_sb[:],
                        start=True, stop=True,
                    )
                    nc.vector.tensor_tensor(
                        attn_sb[:, pt, ic * 512:(ic + 1) * 512],
                        attn_sb[:, pt, ic * 512:(ic + 1) * 512],
                        pb[:],
                        mybir.AluOpType.mult,
                    )

            # ---- phase 4: output projection (partial, this core's heads) ----
            for mt in range(2):
                for icg in range(2):  # groups of 2 i-chunks
                    pp = ps_sc.tile([128, 2, 512], F32, name="ps_sc", tag="ps_sc")
                    for u in range(2):
                        ic = 2 * icg + u
                        for kt in range(2):
                            nc.tensor.matmul(
                                pp[:, u, :],
                                r(wo_sb[:, kt, mt * 128:(mt + 1) * 128]),
                                r(attn_sb[:, kt, ic * 512:(ic + 1) * 512]),
                                start=(kt == 0), stop=(kt == 1),
                            )
                    nc.vector.tensor_copy(
                        out=out_sb[:, mt, icg * 1024:(icg + 1) * 1024].rearrange(
                            "p (a f) -> p a f", a=2),
                        in_=pp[:],
                    )
                    nc.sync.dma_start(
                        out=out_d[mt, :, icg * 1024:(icg + 1) * 1024],
                        in_=out_sb[:, mt, icg * 1024:(icg + 1) * 1024],
                    )

    return nc


def build_nc():
    global _CACHED_NC
    if _CACHED_NC is None:
        # Bacc (not plain Bass): its compile() runs
        # move_matmul_waits_to_ldweights + generate_event_semaphores, which
        # split multi-wait instructions that walrus codegen rejects.
        nc = bacc.Bacc(None)
        _emit(nc)
        if not nc.is_finalized():
            nc.finalize()
        _CACHED_NC = nc
    return _CACHED_NC


def make_core_inputs(x, w_qkv, w_out):
    """Host-side shard prep. Returns list of per-core input dicts."""
    x = np.asarray(x, np.float32)
    w_qkv = np.asarray(w_qkv, np.float32)
    w_out = np.asarray(w_out, np.float32)
    wq = w_qkv[0:HID] * np.float32(SCALE)   # (512, 256), head h rows h*64:...
    wk = w_qkv[HID:2 * HID]
    wv = w_qkv[2 * HID:3 * HID]
    onehot = np.zeros((1, 256), np.float32)
    for i in range(16):
        onehot[0, i * 16 + i] = 1.0
    sel16 = np.zeros((16, 8, 128), np.float32)
    for ic in range(4):
        for pt in range(2):
            for p in range(128):
                sel16[4 * ic + 2 * pt + (p >= 64), ic * 2 + pt, p] = 1.0

    in_maps = []
    for core in range(NCORES):
        b, g = divmod(core, 2)
        hs = slice(g * HPC * DH, (g + 1) * HPC * DH)  # 256 hidden cols/rows
        wqk = np.concatenate([wq[hs], wk[hs]], axis=0)          # (512, 256)
        wqkT = np.ascontiguousarray(wqk.T).reshape(2, 128, 512)
        wvT = np.ascontiguousarray(wv[hs].T).reshape(2, 128, 256)
        woT = np.ascontiguousarray(w_out[:, hs].T).reshape(2, 128, 256)
        in_maps.append({
            "x": np.ascontiguousarray(x[b]).reshape(2, 128, N).astype(np.float16),
            "wqkT": wqkT.astype(np.float16),
            "wvT": wvT.astype(np.float16),
            "woutT": woT.astype(np.float16),
            "onehot": onehot,
            "sel16": sel16,
            "ones": np.ones((128, 64), np.float16),
        })
    return in_maps


def kernel(x, w_qkv, w_out, b_out, trace=False):
    nc = build_nc()
    in_maps = make_core_inputs(x, w_qkv, w_out)
    res = run_bass_kernel_spmd(nc, in_maps, core_ids=list(range(NCORES)),
                               trace=trace)
    out = np.empty((B, C, N), np.float32)
    b_out = np.asarray(b_out, np.float32)
    for b in range(B):
        p0 = np.asarray(res.results[2 * b]["out"]).reshape(C, N)
        p1 = np.asarray(res.results[2 * b + 1]["out"]).reshape(C, N)
        out[b] = p0 + p1 + b_out[:, None]
    if trace:
        return out, res
    return out


# revision 27
# speedup vs baseline: 1.8340x; 1.1562x over previous
"""Trainium2 Bass kernel for a multi-head self-attention block.

Reference computation (shapes hardcoded):
    x: (4, 256, 2048) f32, w_qkv: (1536, 256), w_out: (256, 512), b_out: (256,)
    qkv = w_qkv @ x ; q,k,v heads of dim 64 (8 heads); q *= 64**-0.5
    sim = q^T k per (b, h); attn = softmax(sim, axis=-1)
    out = w_out @ (attn @ v^T rearranged) + b_out

Sharding: 8 cores = 4 batches x 2 head-groups (4 heads each). Each core
computes its batch's qkv projection for its heads, attention, and a partial
output projection over its heads' hidden columns. Host sums the two partial
outputs per batch and adds b_out.

Per-core layout choices:
  - scores are computed TRANSPOSED: S_T[j, i] = sum_d k[d,j] q[d,i] via
    matmul(lhsT=k_head, rhs=q_head) so that E = exp(S_T) (ScalarE, straight
    from PSUM) is directly the moving operand of attn@v.
  - attn@v uses lhsT = [v^T | ones] (128 j-rows x 65 cols) so PSUM row 64
    accumulates the softmax denominator alongside the 64 output dims.
  - v^T is produced directly by the qkv projection (lhsT=x, rhs=w_v^T), no
    transposes anywhere in the kernel.
  - normalization happens after attn@v on (4*64, 2048) values: batched
    reciprocal of the 16 denominator rows, PE ones-matmul broadcast across
    partitions, one tensor_tensor multiply.
All matmuls run in float32r (full PE rate for free dim >= 256, fp32 results).
"""

import numpy as np

import concourse.bass as bass
import concourse.mybir as mybir
import concourse.tile as tile
from concourse import bacc
from concourse.bass_utils import run_bass_kernel_spmd

B, C, N = 4, 256, 2048
HEADS = 8  # total; 4 per core
DH = 64
HID = HEADS * DH  # 512
SCALE = DH ** -0.5
NCORES = 8
HPC = 4  # heads per core

F32 = mybir.dt.float32
F32R = mybir.dt.float32r
F16 = mybir.dt.float16

_CACHED_NC = None


def r(ap):
    """Matmul operand passthrough (fp16 tiles are consumed directly)."""
    return ap


def _emit(nc):
    # ---- DRAM parameters (per-core shard) ----
    x_d = nc.declare_dram_parameter("x", [2, 128, N], F16, isOutput=False)
    wqk_d = nc.declare_dram_parameter("wqkT", [2, 128, 512], F16, isOutput=False)
    wv_d = nc.declare_dram_parameter("wvT", [2, 128, 256], F16, isOutput=False)
    wo_d = nc.declare_dram_parameter("woutT", [2, 128, 256], F16, isOutput=False)
    # one-hot row-gather (1, 4*4) and per-pt partition-broadcast selectors
    oneh_d = nc.declare_dram_parameter("onehot", [1, 16], F16, isOutput=False)
    sel_d = nc.declare_dram_parameter("sel4", [4, 2, 128], F16, isOutput=False)
    ones_d = nc.declare_dram_parameter("ones", [128, 64], F16, isOutput=False)
    out_d = nc.declare_dram_parameter("out", [2, 128, N], F32, isOutput=True)

    EXP = mybir.ActivationFunctionType.Exp

    with tile.TileContext(nc) as tc:
        with (
            tc.tile_pool(name="const", bufs=1) as const,
            tc.tile_pool(name="epool", bufs=3) as epool,
            tc.tile_pool(name="rpool", bufs=2) as rpool,
            tc.tile_pool(name="ps_sc", bufs=2, space="PSUM") as ps_sc,
            tc.tile_pool(name="ps_av", bufs=2, space="PSUM") as ps_av,
            tc.tile_pool(name="ps_misc", bufs=2, space="PSUM") as ps_misc,
        ):
            # ---- persistent SBUF tiles ----
            x_sb = const.tile([128, 2, N], F16)            # x, c on partitions
            wqk_sb = const.tile([128, 2, 512], F16)        # (c, [q|k] rows)
            wv_sb = const.tile([128, 2, 256], F16)         # (c, v^T cols)
            wo_sb = const.tile([128, 2, 256], F16)         # (hd, c_out)
            oneh_sb = const.tile([1, 16], F16)
            sel_sb = const.tile([4, 2, 128], F16)          # bcast selectors
            qk_sb = const.tile([128, 4, N], F16)           # mt: qh01,qh23,kh01,kh23
            vt_sb = const.tile([128, 16, HPC, DH + 1], F16)  # (j, jt, h, d|ones)
            attn_sb = const.tile([128, 2, N], F16)         # attn@v out (hd, i)
            den_free = const.tile([1, 16, 512], F16)       # slot = 4*ic + h
            out_sb = const.tile([128, 2, N], F32)

            for kt in range(2):
                nc.sync.dma_start(out=x_sb[:, kt, :], in_=x_d[kt])
                nc.sync.dma_start(out=wqk_sb[:, kt, :], in_=wqk_d[kt])
                nc.sync.dma_start(out=wv_sb[:, kt, :], in_=wv_d[kt])
                nc.sync.dma_start(out=wo_sb[:, kt, :], in_=wo_d[kt])
            nc.sync.dma_start(out=sel_sb[:], in_=sel_d[:])
            nc.sync.dma_start(out=oneh_sb[:], in_=oneh_d[:])
            # ones column of v^T-augmented tiles
            nc.sync.dma_start(
                out=vt_sb[:, :, :, DH],
                in_=ones_d.rearrange("p (a b) -> p a b", a=16),
            )

            # ---- phase 1: qkv projection ----
            # q,k rows (lhsT reused across the two 512-chunks of each group);
            # emit in [q0, k0, vT, q1, k1] order so pair-0 attention and its
            # exp stream start as early as possible.
            def qk_mtile(mt):
                for icg in range(2):
                    pq = ps_sc.tile([128, 2, 512], F32, name="ps_sc", tag="ps_sc")
                    for kt in range(2):
                        for u in range(2):
                            base = icg * 1024 + u * 512
                            nc.tensor.matmul(
                                pq[:, u, :],
                                wqk_sb[:, kt, mt * 128:(mt + 1) * 128],
                                x_sb[:, kt, base:base + 512],
                                start=(kt == 0), stop=(kt == 1),
                            )
                    nc.vector.tensor_copy(
                        out=qk_sb[:, mt, icg * 1024:(icg + 1) * 1024].rearrange(
                            "p (a f) -> p a f", a=2),
                        in_=pq[:],
                    )

            def v_ntile(nt):
                pv = ps_misc.tile([128, 256], F32, name="ps_misc", tag="ps_misc")
                for kt in range(2):
                    nc.tensor.matmul(
                        pv[:],
                        x_sb[:, kt, nt * 128:(nt + 1) * 128],
                        wv_sb[:, kt, :],
                        start=(kt == 0), stop=(kt == 1),
                    )
                nc.vector.tensor_copy(
                    out=vt_sb[:, nt, :, 0:DH],
                    in_=pv.rearrange("p (h d) -> p h d", h=HPC),
                )

            qk_mtile(0)
            qk_mtile(2)
            for nt in range(16):
                v_ntile(nt)
            qk_mtile(1)
            qk_mtile(3)

            # ---- phase 2-4: attention, fused per i-chunk ----
            for ic in range(4):
                isl = slice(ic * 512, (ic + 1) * 512)
                for pr in range(2):            # head pair (2*pr, 2*pr+1)
                    mt_q, mt_k = pr, 2 + pr
                    pavs = [
                        ps_av.tile([DH + 1, 512], F32, name="ps_av", tag="ps_av")
                        for _ in range(2)
                    ]
                    for half in range(2):
                        e_sb = epool.tile([128, 8, 2, 512], F16, name="e", tag="e")
                        for j in range(8):
                            jt = half * 8 + j
                            psc = ps_sc.tile([128, 2, 512], F32, name="ps_sc",
                                             tag="ps_sc")
                            # the two heads run as concurrent row-group
                            # matmuls (K=64 each) on disjoint PE quadrants
                            for u in range(2):
                                nc.tensor.matmul(
                                    psc[:, u, :],
                                    qk_sb[64 * u:64 * u + 64, mt_k,
                                          jt * 128:(jt + 1) * 128],
                                    qk_sb[64 * u:64 * u + 64, mt_q, isl],
                                    start=True, stop=True,
                                    tile_position=(64 * u, 0),
                                )
                            nc.scalar.activation(e_sb[:, j, :, :], psc[:], EXP)
                        for j in range(8):
                            jt = half * 8 + j
                            for u in range(2):
                                nc.tensor.matmul(
                                    pavs[u][:],
                                    vt_sb[:, jt, 2 * pr + u, :],
                                    e_sb[:, j, u, :],
                                    start=(half == 0 and j == 0),
                                    stop=(half == 1 and j == 7),
                                )
                    for u in range(2):
                        h = 2 * pr + u
                        nc.vector.tensor_copy(
                            out=attn_sb[64 * u:64 * u + 64, pr, isl],
                            in_=pavs[u][0:DH, :],
                        )
                        nc.vector.tensor_copy(
                            out=den_free[0:1, 4 * ic + h, :],
                            in_=pavs[u][DH:DH + 1, :],
                        )

                # normalize + output projection for this i-chunk
                pden = ps_misc.tile([4, 512], F32, name="ps_misc", tag="ps_misc")
                for h in range(4):
                    nc.tensor.matmul(
                        pden[:],
                        oneh_sb[0:1, 4 * h:4 * h + 4],
                        den_free[0:1, 4 * ic + h, :],
                        start=(h == 0), stop=(h == 3),
                    )
                rec4 = rpool.tile([4, 512], F16, name="rec4", tag="rec4")
                with nc.allow_low_precision(
                        reason="fp16 reciprocal feeds fp16 broadcast matmul"):
                    nc.vector.reciprocal(rec4[:], pden[:])
                for pt in range(2):
                    pb = ps_misc.tile([128, 512], F32, name="ps_misc",
                                      tag="ps_misc")
                    nc.tensor.matmul(pb[:], sel_sb[:, pt, :], rec4[:],
                                     start=True, stop=True)
                    nc.vector.tensor_tensor(
                        attn_sb[:, pt, isl], attn_sb[:, pt, isl], pb[:],
                        mybir.AluOpType.mult,
                    )
                for mt in range(2):
                    po = ps_misc.tile([128, 512], F32, name="ps_misc",
                                      tag="ps_misc")
                    for kt in range(2):
                        nc.tensor.matmul(
                            po[:],
                            wo_sb[:, kt, mt * 128:(mt + 1) * 128],
                            attn_sb[:, kt, isl],
                            start=(kt == 0), stop=(kt == 1),
                        )
                    nc.vector.tensor_copy(out=out_sb[:, mt, isl], in_=po[:])
                    nc.sync.dma_start(out=out_d[mt, :, isl],
                                      in_=out_sb[:, mt, isl])

    return nc


def build_nc():
    global _CACHED_NC
    if _CACHED_NC is None:
        # Bacc (not plain Bass): its compile() runs
        # move_matmul_waits_to_ldweights + generate_event_semaphores, which
        # split multi-wait instructions that walrus codegen rejects.
        nc = bacc.Bacc(None)
        _emit(nc)
        if not nc.is_finalized():
            nc.finalize()
        _CACHED_NC = nc
    return _CACHED_NC


def make_core_inputs(x, w_qkv, w_out):
    """Host-side shard prep. Returns list of per-core input dicts."""
    x = np.asarray(x, np.float32)
    w_qkv = np.asarray(w_qkv, np.float32)
    w_out = np.asarray(w_out, np.float32)
    wq = w_qkv[0:HID] * np.float32(SCALE)   # (512, 256), head h rows h*64:...
    wk = w_qkv[HID:2 * HID]
    wv = w_qkv[2 * HID:3 * HID]
    onehot = np.zeros((1, 16), np.float16)
    for h in range(4):
        onehot[0, h * 4 + h] = 1.0
    sel4 = np.zeros((4, 2, 128), np.float16)
    for pt in range(2):
        for p in range(128):
            sel4[2 * pt + (p >= 64), pt, p] = 1.0

    in_maps = []
    for core in range(NCORES):
        b, g = divmod(core, 2)
        hs = slice(g * HPC * DH, (g + 1) * HPC * DH)  # 256 hidden cols/rows
        wqk = np.concatenate([wq[hs], wk[hs]], axis=0)          # (512, 256)
        wqkT = np.ascontiguousarray(wqk.T).reshape(2, 128, 512)
        wvT = np.ascontiguousarray(wv[hs].T).reshape(2, 128, 256)
        woT = np.ascontiguousarray(w_out[:, hs].T).reshape(2, 128, 256)
        in_maps.append({
            "x": np.ascontiguousarray(x[b]).reshape(2, 128, N).astype(np.float16),
            "wqkT": wqkT.astype(np.float16),
            "wvT": wvT.astype(np.float16),
            "woutT": woT.astype(np.float16),
            "onehot": onehot,
            "sel4": sel4,
            "ones": np.ones((128, 64), np.float16),
        })
    return in_maps


def kernel(x, w_qkv, w_out, b_out, trace=False):
    nc = build_nc()
    in_maps = make_core_inputs(x, w_qkv, w_out)
    res = run_bass_kernel_spmd(nc, in_maps, core_ids=list(range(NCORES)),
                               trace=trace)
    out = np.empty((B, C, N), np.float32)
    b_out = np.asarray(b_out, np.float32)
    for b in range(B):
        p0 = np.asarray(res.results[2 * b]["out"]).reshape(C, N)
        p1 = np.asarray(res.results[2 * b + 1]["out"]).reshape(C, N)
        out[b] = p0 + p1 + b_out[:, None]
    if trace:
        return out, res
    return out


# revision 33
# speedup vs baseline: 1.8689x; 1.0190x over previous
"""Trainium2 Bass kernel for a multi-head self-attention block.

Reference computation (shapes hardcoded):
    x: (4, 256, 2048) f32, w_qkv: (1536, 256), w_out: (256, 512), b_out: (256,)
    qkv = w_qkv @ x ; q,k,v heads of dim 64 (8 heads); q *= 64**-0.5
    sim = q^T k per (b, h); attn = softmax(sim, axis=-1)
    out = w_out @ (attn @ v^T rearranged) + b_out

Sharding: 8 cores = 4 batches x 2 head-groups (4 heads each). Each core
computes its batch's qkv projection for its heads, attention, and a partial
output projection over its heads' hidden columns. Host sums the two partial
outputs per batch and adds b_out.

Per-core layout choices:
  - scores are computed TRANSPOSED: S_T[j, i] = sum_d k[d,j] q[d,i] via
    matmul(lhsT=k_head, rhs=q_head) so that E = exp(S_T) (ScalarE, straight
    from PSUM) is directly the moving operand of attn@v.
  - attn@v uses lhsT = [v^T | ones] (128 j-rows x 65 cols) so PSUM row 64
    accumulates the softmax denominator alongside the 64 output dims.
  - v^T is produced directly by the qkv projection (lhsT=x, rhs=w_v^T), no
    transposes anywhere in the kernel.
  - normalization happens after attn@v on (4*64, 2048) values: batched
    reciprocal of the 16 denominator rows, PE ones-matmul broadcast across
    partitions, one tensor_tensor multiply.
All matmuls run in float32r (full PE rate for free dim >= 256, fp32 results).
"""

import numpy as np

import concourse.bass as bass
import concourse.mybir as mybir
import concourse.tile as tile
from concourse import bacc
from concourse.bass_utils import run_bass_kernel_spmd

B, C, N = 4, 256, 2048
HEADS = 8  # total; 4 per core
DH = 64
HID = HEADS * DH  # 512
SCALE = DH ** -0.5
NCORES = 8
HPC = 4  # heads per core

F32 = mybir.dt.float32
F32R = mybir.dt.float32r
F16 = mybir.dt.float16

_CACHED_NC = None


def r(ap):
    """Matmul operand passthrough (fp16 tiles are consumed directly)."""
    return ap


def _emit(nc):
    # ---- DRAM parameters (per-core shard) ----
    x_d = nc.declare_dram_parameter("x", [2, 128, N], F16, isOutput=False)
    wqk_d = nc.declare_dram_parameter("wqkT", [2, 128, 512], F16, isOutput=False)
    wv_d = nc.declare_dram_parameter("wvT", [2, 128, 256], F16, isOutput=False)
    wo_d = nc.declare_dram_parameter("woutT", [2, 128, 256], F16, isOutput=False)
    # one-hot row-gather (1, 4*4) and per-pt partition-broadcast selectors
    oneh_d = nc.declare_dram_parameter("onehot", [1, 16], F16, isOutput=False)
    sel_d = nc.declare_dram_parameter("sel4", [4, 2, 128], F16, isOutput=False)
    ones_d = nc.declare_dram_parameter("ones", [128, 64], F16, isOutput=False)
    out_d = nc.declare_dram_parameter("out", [2, 128, N], F32, isOutput=True)

    EXP = mybir.ActivationFunctionType.Exp

    with tile.TileContext(nc) as tc:
        with (
            tc.tile_pool(name="const", bufs=1) as const,
            tc.tile_pool(name="epool", bufs=3) as epool,
            tc.tile_pool(name="rpool", bufs=2) as rpool,
            tc.tile_pool(name="ps_sc", bufs=2, space="PSUM") as ps_sc,
            tc.tile_pool(name="ps_av", bufs=4, space="PSUM") as ps_av,
        ):
            # ---- persistent SBUF tiles ----
            x_sb = const.tile([128, 2, N], F16)            # x, c on partitions
            wqk_sb = const.tile([128, 2, 512], F16)        # (c, [q|k] rows)
            wv_sb = const.tile([128, 2, 256], F16)         # (c, v^T cols)
            wo_sb = const.tile([128, 2, 256], F16)         # (hd, c_out)
            oneh_sb = const.tile([1, 16], F16)
            sel_sb = const.tile([4, 2, 128], F16)          # bcast selectors
            qk_sb = const.tile([128, 4, N], F16)           # mt: qh01,qh23,kh01,kh23
            vt_sb = const.tile([128, 16, HPC, DH + 1], F16)  # (j, jt, h, d|ones)
            attn_sb = const.tile([128, 2, N], F16)         # attn@v out (hd, i)
            den_free = const.tile([1, 16, 512], F16)       # slot = 4*ic + h
            out_sb = const.tile([128, 2, N], F32)

            for kt in range(2):
                nc.sync.dma_start(out=x_sb[:, kt, :], in_=x_d[kt])
                nc.sync.dma_start(out=wqk_sb[:, kt, :], in_=wqk_d[kt])
                nc.sync.dma_start(out=wv_sb[:, kt, :], in_=wv_d[kt])
                nc.sync.dma_start(out=wo_sb[:, kt, :], in_=wo_d[kt])
            nc.sync.dma_start(out=sel_sb[:], in_=sel_d[:])
            nc.sync.dma_start(out=oneh_sb[:], in_=oneh_d[:])
            # ones column of v^T-augmented tiles
            nc.sync.dma_start(
                out=vt_sb[:, :, :, DH],
                in_=ones_d.rearrange("p (a b) -> p a b", a=16),
            )

            # ---- phase 1: qkv projection ----
            # q,k rows (lhsT reused across the two 512-chunks of each group);
            # emit in [q0, k0, vT, q1, k1] order so pair-0 attention and its
            # exp stream start as early as possible.
            def qk_mtile(mt):
                for icg in range(2):
                    pq = ps_sc.tile([128, 2, 512], F32, name="ps_sc", tag="ps_sc")
                    for kt in range(2):
                        for u in range(2):
                            base = icg * 1024 + u * 512
                            nc.tensor.matmul(
                                pq[:, u, :],
                                wqk_sb[:, kt, mt * 128:(mt + 1) * 128],
                                x_sb[:, kt, base:base + 512],
                                start=(kt == 0), stop=(kt == 1),
                            )
                    nc.vector.tensor_copy(
                        out=qk_sb[:, mt, icg * 1024:(icg + 1) * 1024].rearrange(
                            "p (a f) -> p a f", a=2),
                        in_=pq[:],
                    )

            def v_ntile(nt):
                pv = ps_av.tile([128, 256], F32, name="ps_av", tag="ps_av")
                for kt in range(2):
                    nc.tensor.matmul(
                        pv[:],
                        x_sb[:, kt, nt * 128:(nt + 1) * 128],
                        wv_sb[:, kt, :],
                        start=(kt == 0), stop=(kt == 1),
                    )
                nc.vector.tensor_copy(
                    out=vt_sb[:, nt, :, 0:DH],
                    in_=pv.rearrange("p (h d) -> p h d", h=HPC),
                )

            def scores_exp(ic, pr, half):
                """16 paired score matmuls + 8 exps -> one E half-tile."""
                isl = slice(ic * 512, (ic + 1) * 512)
                mt_q, mt_k = pr, 2 + pr
                e_sb = epool.tile([128, 8, 2, 512], F16, name="e", tag="e")
                for j in range(8):
                    jt = half * 8 + j
                    psc = ps_sc.tile([128, 2, 512], F32, name="ps_sc",
                                     tag="ps_sc")
                    # the two heads run as concurrent row-group matmuls
                    # (K=64 each) on disjoint PE quadrants, sharing the
                    # q-column stream
                    for u in range(2):
                        nc.tensor.matmul(
                            psc[:, u, :],
                            qk_sb[64 * u:64 * u + 64, mt_k,
                                  jt * 128:(jt + 1) * 128],
                            qk_sb[64 * u:64 * u + 64, mt_q, isl],
                            start=True, stop=True,
                            tile_position=(64 * u, 0),
                        )
                    nc.scalar.activation(e_sb[:, j, :, :], psc[:], EXP)
                return e_sb

            def attnv_half(pavs, pr, half, e_sb):
                for j in range(8):
                    jt = half * 8 + j
                    for u in range(2):
                        nc.tensor.matmul(
                            pavs[u][:],
                            vt_sb[:, jt, 2 * pr + u, :],
                            e_sb[:, j, u, :],
                            start=(half == 0 and j == 0),
                            stop=(half == 1 and j == 7),
                        )

            def pair_drain(pavs, ic, pr):
                isl = slice(ic * 512, (ic + 1) * 512)
                for u in range(2):
                    h = 2 * pr + u
                    nc.vector.tensor_copy(
                        out=attn_sb[64 * u:64 * u + 64, pr, isl],
                        in_=pavs[u][0:DH, :],
                    )
                    nc.vector.tensor_copy(
                        out=den_free[0:1, 4 * ic + h, :],
                        in_=pavs[u][DH:DH + 1, :],
                    )

            def alloc_pavs():
                return [
                    ps_av.tile([DH + 1, 512], F32, name="ps_av", tag="ps_av")
                    for _ in range(2)
                ]

            def ic_tail(ic):
                """denominator gather + reciprocal + broadcast + normalize +
                output projection for one i-chunk."""
                isl = slice(ic * 512, (ic + 1) * 512)
                pden = ps_av.tile([4, 512], F32, name="ps_av", tag="ps_av")
                for h in range(4):
                    nc.tensor.matmul(
                        pden[:],
                        oneh_sb[0:1, 4 * h:4 * h + 4],
                        den_free[0:1, 4 * ic + h, :],
                        start=(h == 0), stop=(h == 3),
                    )
                rec4 = rpool.tile([4, 512], F16, name="rec4", tag="rec4")
                with nc.allow_low_precision(
                        reason="fp16 reciprocal feeds fp16 broadcast matmul"):
                    nc.vector.reciprocal(rec4[:], pden[:])
                for pt in range(2):
                    pb = ps_av.tile([128, 512], F32, name="ps_av", tag="ps_av")
                    nc.tensor.matmul(pb[:], sel_sb[:, pt, :], rec4[:],
                                     start=True, stop=True)
                    nc.vector.tensor_tensor(
                        attn_sb[:, pt, isl], attn_sb[:, pt, isl], pb[:],
                        mybir.AluOpType.mult,
                    )
                for mt in range(2):
                    po = ps_av.tile([128, 512], F32, name="ps_av", tag="ps_av")
                    for kt in range(2):
                        nc.tensor.matmul(
                            po[:],
                            wo_sb[:, kt, mt * 128:(mt + 1) * 128],
                            attn_sb[:, kt, isl],
                            start=(kt == 0), stop=(kt == 1),
                        )
                    nc.vector.tensor_copy(out=out_sb[:, mt, isl], in_=po[:])
                    nc.sync.dma_start(out=out_d[mt, :, isl],
                                      in_=out_sb[:, mt, isl])

            # phase 1 + warm start: k/q for pair 0 first, then kick off the
            # (ic0, pr0) exp stream while v^T and the pair-1 projections run
            qk_mtile(2)
            qk_mtile(0)
            e00 = scores_exp(0, 0, 0)
            e01 = scores_exp(0, 0, 1)
            for nt in range(16):
                v_ntile(nt)
            qk_mtile(3)
            qk_mtile(1)

            for ic in range(4):
                for pr in range(2):
                    pavs = alloc_pavs()
                    if ic == 0 and pr == 0:
                        attnv_half(pavs, pr, 0, e00)
                        attnv_half(pavs, pr, 1, e01)
                    else:
                        for half in range(2):
                            e_sb = scores_exp(ic, pr, half)
                            attnv_half(pavs, pr, half, e_sb)
                    pair_drain(pavs, ic, pr)
                ic_tail(ic)

    return nc


def build_nc():
    global _CACHED_NC
    if _CACHED_NC is None:
        # Bacc (not plain Bass): its compile() runs
        # move_matmul_waits_to_ldweights + generate_event_semaphores, which
        # split multi-wait instructions that walrus codegen rejects.
        nc = bacc.Bacc(None)
        _emit(nc)
        if not nc.is_finalized():
            nc.finalize()
        _CACHED_NC = nc
    return _CACHED_NC


def make_core_inputs(x, w_qkv, w_out):
    """Host-side shard prep. Returns list of per-core input dicts."""
    x = np.asarray(x, np.float32)
    w_qkv = np.asarray(w_qkv, np.float32)
    w_out = np.asarray(w_out, np.float32)
    wq = w_qkv[0:HID] * np.float32(SCALE)   # (512, 256), head h rows h*64:...
    wk = w_qkv[HID:2 * HID]
    wv = w_qkv[2 * HID:3 * HID]
    onehot = np.zeros((1, 16), np.float16)
    for h in range(4):
        onehot[0, h * 4 + h] = 1.0
    sel4 = np.zeros((4, 2, 128), np.float16)
    for pt in range(2):
        for p in range(128):
            sel4[2 * pt + (p >= 64), pt, p] = 1.0

    in_maps = []
    for core in range(NCORES):
        b, g = divmod(core, 2)
        hs = slice(g * HPC * DH, (g + 1) * HPC * DH)  # 256 hidden cols/rows
        wqk = np.concatenate([wq[hs], wk[hs]], axis=0)          # (512, 256)
        wqkT = np.ascontiguousarray(wqk.T).reshape(2, 128, 512)
        wvT = np.ascontiguousarray(wv[hs].T).reshape(2, 128, 256)
        woT = np.ascontiguousarray(w_out[:, hs].T).reshape(2, 128, 256)
        in_maps.append({
            "x": np.ascontiguousarray(x[b]).reshape(2, 128, N).astype(np.float16),
            "wqkT": wqkT.astype(np.float16),
            "wvT": wvT.astype(np.float16),
            "woutT": woT.astype(np.float16),
            "onehot": onehot,
            "sel4": sel4,
            "ones": np.ones((128, 64), np.float16),
        })
    return in_maps


def kernel(x, w_qkv, w_out, b_out, trace=False):
    nc = build_nc()
    in_maps = make_core_inputs(x, w_qkv, w_out)
    res = run_bass_kernel_spmd(nc, in_maps, core_ids=list(range(NCORES)),
                               trace=trace)
    out = np.empty((B, C, N), np.float32)
    b_out = np.asarray(b_out, np.float32)
    for b in range(B):
        p0 = np.asarray(res.results[2 * b]["out"]).reshape(C, N)
        p1 = np.asarray(res.results[2 * b + 1]["out"]).reshape(C, N)
        out[b] = p0 + p1 + b_out[:, None]
    if trace:
        return out, res
    return out
